# revision 1
# baseline (speedup 1.0000x reference)
"""Multi-head attention (b=2, n=2048, d_model=1024, H=16, d_k=d_v=64) on 8
Trainium2 NeuronCores.

Sharding: 8 cores = 2 (batch) x 4 (head groups of 4 heads).  Each core
computes, for its batch ib and head group g:

    q/k projections in transposed form  qT = Wq_g @ x^T   [256, 2048]
    v projection in natural form        V  = x @ Wv_g^T   [2048, 256]
    per head: S^T = K_h Q_h^T (k on partitions), A^T = exp(S^T/8),
              O^T|Z = [V_h|1]^T A^T  (PE row 64 gives softmax denom Z)
    normalize O^T by 1/Z, out-projection Y^T = Wo_g @ O_cat^T  [1024, 2048]

Host sums the 4 per-group partial Y^T per batch and adds bo.

All matmuls run as float32r (TF32-like, ~1.6e-4 relative error measured on
hw).  Softmax skips the max-subtraction: scores*scale are ~N(0,1) so exp
never overflows in fp32.
"""

import numpy as np
from contextlib import ExitStack

import concourse.bass as bass
import concourse.mybir as mybir
import concourse.tile as tile
from concourse import bacc
from concourse.bass_utils import run_bass_kernel_spmd

F32 = mybir.dt.float32
F32R = mybir.dt.float32r
EXP = mybir.ActivationFunctionType.Exp
ADD = mybir.AluOpType.add
MULT = mybir.AluOpType.mult

D_MODEL = 1024
H = 16
DK = 64
B = 2
N = 2048           # nq = nk
G = 4              # head groups (cores per batch)
HG = H // G        # heads per group = 4
DG = HG * DK       # 256 group dims
KT = 8             # D_MODEL / 128 contraction tiles
NKT = N // 128     # 16 k-tiles in attention
QC = 1024          # attention q-chunk
NCH = N // QC      # 2 chunks
P = 128

_PROGRAM = None


def _build_program():
    nc = bacc.Bacc("TRN2", target_bir_lowering=False, debug=False, num_devices=8)

    # pre-tiled on host: [n-chunk, p, k, 512] so each DMA partition line is
    # one fully contiguous 16 KiB read
    xqT = nc.dram_tensor("xqT", [4, P, KT, 512], F32, kind="ExternalInput").ap()
    xkT = nc.dram_tensor("xkT", [4, P, KT, 512], F32, kind="ExternalInput").ap()
    xvT = nc.dram_tensor("xvT", [NKT, P, KT, P], F32, kind="ExternalInput").ap()
    wqT = nc.dram_tensor("wqT", [P, KT, DG], F32, kind="ExternalInput").ap()
    wkT = nc.dram_tensor("wkT", [P, KT, DG], F32, kind="ExternalInput").ap()
    wvT = nc.dram_tensor("wvT", [P, KT, DG], F32, kind="ExternalInput").ap()
    woT = nc.dram_tensor("woT", [P, 2, D_MODEL], F32, kind="ExternalInput").ap()
    bq_d = nc.dram_tensor("bq_s", [DG], F32, kind="ExternalInput").ap()
    bk_d = nc.dram_tensor("bk_s", [DG], F32, kind="ExternalInput").ap()
    bv_d = nc.dram_tensor("bv_s", [DG], F32, kind="ExternalInput").ap()
    ones_d = nc.dram_tensor("ones_c", [P], F32, kind="ExternalInput").ap()
    yT_d = nc.dram_tensor("yT", [D_MODEL, N], F32, kind="ExternalOutput").ap()
    # dram staging for softmax denominators (internal DRAM tiles fail to load
    # under the axon PJRT path, so an ExternalOutput buffer instead)
    z_st = nc.dram_tensor("z_st", [16, 512], F32, kind="ExternalOutput").ap()

    bq_v = bq_d.rearrange("(j p) -> p j", p=P)        # [128, 2]
    bk_v = bk_d.rearrange("(j p) -> p j", p=P)

    with tile.TileContext(nc) as tc:
        with ExitStack() as ctx:
            const = ctx.enter_context(tc.tile_pool(name="const", bufs=1))
            xin = ctx.enter_context(tc.tile_pool(name="xin", bufs=2))
            xvp = ctx.enter_context(tc.tile_pool(name="xvp", bufs=3))
            work = ctx.enter_context(tc.tile_pool(name="work", bufs=2))
            atp = ctx.enter_context(tc.tile_pool(name="atp", bufs=4))
            smal = ctx.enter_context(tc.tile_pool(name="smal", bufs=4))
            big = ctx.enter_context(tc.tile_pool(name="big", bufs=2, space="PSUM"))
            avp = ctx.enter_context(tc.tile_pool(name="avp", bufs=4, space="PSUM"))

            # ---- constants (weights on the ACT HWDGE queue so they overlap
            # the first x chunks on the SP queue) ----
            wq_sb = const.tile([P, KT, DG], F32R, tag="wq")
            wk_sb = const.tile([P, KT, DG], F32R, tag="wk")
            wv_sb = const.tile([P, KT, DG], F32R, tag="wv")
            wo_sb = const.tile([P, 2, D_MODEL], F32R, tag="wo")
            nc.scalar.dma_start(wk_sb[:], wkT.bitcast(F32R))
            nc.scalar.dma_start(wv_sb[:], wvT.bitcast(F32R))
            nc.scalar.dma_start(wq_sb[:], wqT.bitcast(F32R))
            nc.scalar.dma_start(wo_sb[:], woT.bitcast(F32R))
            bq_sb = const.tile([P, 2], F32, tag="bq")
            bk_sb = const.tile([P, 2], F32, tag="bk")
            nc.scalar.dma_start(bq_sb[:], bq_v)
            nc.scalar.dma_start(bk_sb[:], bk_v)
            bv_sb = const.tile([1, DG], F32R, tag="bv")
            nc.scalar.dma_start(bv_sb[:], bv_d[None, :].bitcast(F32R))
            ones_sb = const.tile([1, P], F32R, tag="ones")
            nc.scalar.dma_start(ones_sb[:], ones_d[None, :].bitcast(F32R))

            kt_sb = const.tile([P, 2, N], F32R, tag="kt")          # K^T, d' on part
            v_sb = const.tile([P, NKT, HG, DK + 1], F32R, tag="v")  # [V_h | 1]
            nc.scalar.dma_start(
                v_sb[:, :, :, DK].rearrange("p a b -> p (a b)"),
                ones_d[:, None].to_broadcast((P, NKT * HG)).bitcast(F32R),
            )

            def qt_proj(c, qt):
                for qh in range(2):
                    xq = xin.tile([P, KT, 512], F32R, tag="xchunk",
                                  name=f"xq_{c}_{qh}")
                    nc.sync.dma_start(xq[:], xqT[c * 2 + qh].bitcast(F32R))
                    for j in range(2):
                        ps = big.tile([P, 512], F32, tag="big",
                                      name=f"qps_{c}_{qh}_{j}")
                        for k in range(KT):
                            nc.tensor.matmul(
                                ps[:], wq_sb[:, k, j * P:(j + 1) * P], xq[:, k, :],
                                start=(k == 0), stop=(k == KT - 1))
                        nc.vector.tensor_tensor(
                            qt[:, j, qh * 512:(qh + 1) * 512], ps[:],
                            bq_sb[:, j, None].to_broadcast((P, 512)), ADD)

            def kt_proj(c4):
                xk = xin.tile([P, KT, 512], F32R, tag="xchunk", name=f"xk_{c4}")
                nc.sync.dma_start(xk[:], xkT[c4].bitcast(F32R))
                for j in range(2):
                    ps = big.tile([P, 512], F32, tag="big", name=f"kps_{c4}_{j}")
                    for k in range(KT):
                        nc.tensor.matmul(
                            ps[:], wk_sb[:, k, j * P:(j + 1) * P], xk[:, k, :],
                            start=(k == 0), stop=(k == KT - 1))
                    nc.vector.tensor_tensor(
                        kt_sb[:, j, c4 * 512:(c4 + 1) * 512], ps[:],
                        bk_sb[:, j, None].to_broadcast((P, 512)), ADD)

            def v_proj(nt):
                xv = xvp.tile([P, KT, P], F32R, tag="xv", name=f"xv_{nt}")
                nc.sync.dma_start(xv[:], xvT[nt].bitcast(F32R))
                ps = avp.tile([P, DG], F32, tag="av", name=f"vps_{nt}")
                for k in range(KT):
                    nc.tensor.matmul(ps[:], xv[:, k, :], wv_sb[:, k, :],
                                     start=(k == 0), stop=False)
                nc.tensor.matmul(ps[:], ones_sb[:], bv_sb[:], start=False, stop=True)
                nc.vector.tensor_copy(
                    v_sb[:, nt, :, 0:DK],
                    ps[:].rearrange("p (h d) -> p h d", h=HG))

            # ---- phase A: kT then V projections ----
            for c4 in range(4):
                kt_proj(c4)
            for nt in range(NKT):
                v_proj(nt)

            qts = {}
            for c in range(NCH):
                qts[c] = work.tile([P, 2, QC], F32R, tag="qt", name=f"qt_{c}")
            qt_proj(0, qts[0])

            for c in range(NCH):
                qt = qts[c]
                o_sb = work.tile([P, 2, QC], F32R, tag="o", name=f"o_{c}")

                for pair in range(2):  # heads (2*pair, 2*pair+1); j = pair
                    avs = [avp.tile([DK + 1, 512], F32, tag="av", name=f"av_{c}_{pair}_{i}")
                           for i in range(4)]
                    for kt in range(NKT):
                        for hp in range(2):
                            h = 2 * pair + hp
                            p0 = 64 * hp
                            st = big.tile([P, QC], F32, tag="big")
                            for qh in range(2):
                                nc.tensor.matmul(
                                    st[:, qh * 512:(qh + 1) * 512],
                                    kt_sb[p0:p0 + 64, pair, kt * P:(kt + 1) * P],
                                    qt[p0:p0 + 64, pair, qh * 512:(qh + 1) * 512],
                                    start=True, stop=True)
                            at = atp.tile([P, QC], F32R, tag="at")
                            nc.scalar.activation(at[:], st[:], EXP, scale=0.125)
                            for qh in range(2):
                                nc.tensor.matmul(
                                    avs[2 * hp + qh], v_sb[:, kt, h, :],
                                    at[:, qh * 512:(qh + 1) * 512],
                                    start=(kt == 0), stop=(kt == NKT - 1))

                    # overlap the z roundtrip of the last pair with the
                    # next chunk's qT projection on the PE stream
                    if pair == 1 and c + 1 < NCH:
                        qt_proj(c + 1, qts[c + 1])

                    # softmax denominators for this pair: 4 rows of 512.
                    # Broadcast Z via DRAM, then reciprocal_approx_fast on the
                    # broadcast tile (64 lanes) -- short latency chain.
                    r0 = (c * 2 + pair) * 4
                    z_dram = z_st[r0:r0 + 4, :]
                    for i in range(4):
                        zr = smal.tile([1, 512], F32, tag="zrow")
                        nc.vector.tensor_copy(zr[:], avs[i][DK:DK + 1, :])
                        nc.scalar.dma_start(z_dram[i:i + 1, :], zr[:])

                    for hp in range(2):
                        for qh in range(2):
                            zb = smal.tile([64, 512], F32, tag="zb")
                            nc.scalar.dma_start(
                                zb[:],
                                z_dram[2 * hp + qh, None, :].to_broadcast((64, 512)))
                            rzb = smal.tile([64, 512], F32, tag="rzb")
                            nc.vector.reciprocal_approx_fast(rzb[:], zb[:])
                            p0 = 64 * hp
                            nc.vector.tensor_tensor(
                                o_sb[p0:p0 + 64, pair, qh * 512:(qh + 1) * 512],
                                avs[2 * hp + qh][0:DK, :], rzb[:], MULT)

                # out-projection for this chunk: Y^T = Wo_g @ O_cat^T
                for m in range(8):
                    y_sb = smal.tile([P, QC], F32, tag="y")
                    for qh in range(2):
                        yps = big.tile([P, 512], F32, tag="big")
                        for j in range(2):
                            nc.tensor.matmul(
                                yps[:], wo_sb[:, j, m * P:(m + 1) * P],
                                o_sb[:, j, qh * 512:(qh + 1) * 512],
                                start=(j == 0), stop=(j == 1))
                        nc.vector.tensor_copy(y_sb[:, qh * 512:(qh + 1) * 512], yps[:])
                    nc.sync.dma_start(
                        yT_d[m * P:(m + 1) * P, c * QC:(c + 1) * QC], y_sb[:])

    nc.compile()
    return nc


def get_program():
    global _PROGRAM
    if _PROGRAM is None:
        _PROGRAM = _build_program()
    return _PROGRAM


def _tile_xT(x, nchunk, width):
    # x [n, 1024] -> x^T tiled [nchunk, 128 p, 8 k, width]
    xt = np.ascontiguousarray(x.T)                      # [1024, n]
    return np.ascontiguousarray(
        xt.reshape(KT, P, nchunk, width).transpose(2, 1, 0, 3))


def _tile_w(w_rows):
    # w_rows [256, 1024] (= W[g-slice]) -> W^T tiled [128 p, 8 k, 256]
    return np.ascontiguousarray(w_rows.T.reshape(KT, P, DG).transpose(1, 0, 2))


def make_in_maps(queries, keys, values, Wq, bq, Wk, bk, Wv, bv, Wo, bo):
    """Build per-core input dicts. Core c handles batch c//4, head group c%4."""
    f32 = np.float32
    xT = {}
    for ib in range(B):
        xT[ib] = (
            _tile_xT(np.asarray(queries[ib], f32), 4, 512),
            _tile_xT(np.asarray(keys[ib], f32), 4, 512),
            _tile_xT(np.asarray(values[ib], f32), NKT, P),
        )
    ones = np.ones((P,), f32)
    in_maps = []
    for core in range(8):
        ib, g = core // G, core % G
        sl = slice(g * DG, (g + 1) * DG)
        in_maps.append({
            "xqT": xT[ib][0], "xkT": xT[ib][1], "xvT": xT[ib][2],
            "wqT": _tile_w(Wq[sl, :]),
            "wkT": _tile_w(Wk[sl, :]),
            "wvT": _tile_w(Wv[sl, :]),
            "woT": np.ascontiguousarray(
                Wo[:, sl].T.reshape(2, P, D_MODEL).transpose(1, 0, 2)),
            "bq_s": np.ascontiguousarray(bq[sl]),
            "bk_s": np.ascontiguousarray(bk[sl]),
            "bv_s": np.ascontiguousarray(bv[sl]),
            "ones_c": ones,
        })
    return in_maps


def gather_output(results, bo):
    out = np.zeros((B, N, D_MODEL), np.float32)
    for core in range(8):
        out[core // G] += results[core]["yT"].T
    out += bo[None, None, :].astype(np.float32)
    return out


def _run(inputs, trace=False, **spmd_kwargs):
    nc = get_program()
    in_maps = make_in_maps(**inputs)
    res = run_bass_kernel_spmd(nc, in_maps, core_ids=list(range(8)),
                               trace=trace, **spmd_kwargs)
    return gather_output(res.results, inputs["bo"]), res


def kernel(**inputs) -> np.ndarray:
    out, _ = _run(inputs, trace=False)
    return out



# revision 26
# speedup vs baseline: 1.2908x; 1.2908x over previous
"""Multi-head attention (b=2, n=2048, d_model=1024, H=16, d_k=d_v=64) on 8
Trainium2 NeuronCores.

Sharding: 8 cores = 2 (batch) x 4 (head groups of 4 heads).  Each core
computes, for its batch ib and head group g (heads as 2 pairs x 2 hp):

    qT/kT projections   qT = Wq_g @ x^T            [256, 2048]
    v projection        V  = x @ Wv_g^T            [2048, 256]
    per head: S^T = K_h Q_h^T (kpos on partitions), A^T = exp(S^T/8),
              O^T|Z = [V_h|1]^T A^T  (PSUM row 64 gives softmax denom Z)
    normalize O^T by 1/Z (Z broadcast across partitions via a tiny
    ones-outer-product matmul on the PE -- no DRAM roundtrip),
    out-projection Y^T = Wo_g @ O_cat^T            [1024, 2048]

Host sums the 4 per-group partial Y^T per batch and adds bo + Wo@bv
(the V bias commutes through normalized attention; the K bias is a
per-query constant shift of the logits, to which softmax is exactly
invariant, so both are folded out of the device program).

All matmul operands are bf16 (PSUM accumulation stays fp32): this halves
input DMA, enables Fast Weight Load on the PE (2x faster LDWEIGHTS), and
doubles DVE throughput where it applies.  exp is evaluated on fp32 PSUM
logits.  Softmax skips max-subtraction: scores*scale are ~N(0,1).

The PE instruction stream is software-pipelined: within the attention
inner loop the AV matmuls lag the S matmuls by one k-tile so the ACT
(exp) stream is hidden, and projection / out-projection matmuls are
sprinkled between attention tiles as filler so the PE never idles long
enough for the HAM clock gate to re-throttle.
"""

import numpy as np
from contextlib import ExitStack

import ml_dtypes

import concourse.bass as bass
import concourse.mybir as mybir
import concourse.tile as tile
from concourse import bacc
from concourse.bass_utils import run_bass_kernel_spmd

F32 = mybir.dt.float32
BF16 = mybir.dt.bfloat16
NP_BF16 = ml_dtypes.bfloat16
EXP = mybir.ActivationFunctionType.Exp
ADD = mybir.AluOpType.add
MULT = mybir.AluOpType.mult

D_MODEL = 1024
H = 16
DK = 64
B = 2
N = 2048           # nq = nk
G = 4              # head groups (cores per batch)
HG = H // G        # heads per group = 4
DG = HG * DK       # 256 group dims
KT = 8             # D_MODEL / 128 contraction tiles
NKT = N // 128     # 16 k-tiles in attention
QB = 512           # attention q-block
NQB = N // QB      # 4 q-blocks
P = 128

_PROGRAM = None
DEBUG_DUMP = False


def _build_program():
    nc = bacc.Bacc("TRN2", target_bir_lowering=False, debug=False, num_devices=8)

    # pre-tiled on host (bf16): per-partition lines are contiguous reads
    xqT = nc.dram_tensor("xqT", [NQB, P, KT, QB], BF16, kind="ExternalInput").ap()
    xkT = nc.dram_tensor("xkT", [NQB, P, KT, QB], BF16, kind="ExternalInput").ap()
    xvT = nc.dram_tensor("xvT", [NKT, P, KT, P], BF16, kind="ExternalInput").ap()
    wqT = nc.dram_tensor("wqT", [P, KT, DG], BF16, kind="ExternalInput").ap()
    wkT = nc.dram_tensor("wkT", [P, KT, DG], BF16, kind="ExternalInput").ap()
    wvT = nc.dram_tensor("wvT", [P, KT, DG], BF16, kind="ExternalInput").ap()
    woT = nc.dram_tensor("woT", [P, 2, D_MODEL], BF16, kind="ExternalInput").ap()
    bq_d = nc.dram_tensor("bq_s", [DG], F32, kind="ExternalInput").ap()
    ones_d = nc.dram_tensor("ones_c", [P], BF16, kind="ExternalInput").ap()
    yT_d = nc.dram_tensor("yT", [D_MODEL, N], F32, kind="ExternalOutput").ap()
    if DEBUG_DUMP:
        dbg_kt = nc.dram_tensor("dbg_kt", [P, 2, N], F32, kind="ExternalOutput").ap()
        dbg_v = nc.dram_tensor("dbg_v", [P, NKT, HG, DK + 1], F32,
                               kind="ExternalOutput").ap()
        dbg_qt = nc.dram_tensor("dbg_qt", [P, 2, QB], F32, kind="ExternalOutput").ap()
        dbg_o = nc.dram_tensor("dbg_o", [P, 2, QB], F32, kind="ExternalOutput").ap()
        dbg_at = nc.dram_tensor("dbg_at", [P, 2, QB], F32, kind="ExternalOutput").ap()
        dbg_av = nc.dram_tensor("dbg_av", [P, QB], F32, kind="ExternalOutput").ap()
        dbg_bc = nc.dram_tensor("dbg_bc", [P, QB], F32, kind="ExternalOutput").ap()

    bq_v = bq_d.rearrange("(j p) -> p j", p=P)        # [128, 2]

    with tile.TileContext(nc) as tc:
        with ExitStack() as ctx:
            const = ctx.enter_context(tc.tile_pool(name="const", bufs=1))
            xin = ctx.enter_context(tc.tile_pool(name="xin", bufs=2))
            xvp = ctx.enter_context(tc.tile_pool(name="xvp", bufs=3))
            atp = ctx.enter_context(tc.tile_pool(name="atp", bufs=3))
            smal = ctx.enter_context(tc.tile_pool(name="smal", bufs=2))
            stp = ctx.enter_context(tc.tile_pool(name="stp", bufs=2, space="PSUM"))
            avp = ctx.enter_context(tc.tile_pool(name="avp", bufs=2, space="PSUM"))
            aux = ctx.enter_context(tc.tile_pool(name="aux", bufs=2, space="PSUM"))

            # ---- constants: weights on the ACT queue (idle at start),
            # x chunks on SP, xv on DVE ----
            wk_sb = const.tile([P, KT, DG], BF16, tag="wk")
            wv_sb = const.tile([P, KT, DG], BF16, tag="wv")
            wq_sb = const.tile([P, KT, DG], BF16, tag="wq")
            wo_sb = const.tile([P, 2, D_MODEL], BF16, tag="wo")
            nc.scalar.dma_start(wk_sb[:], wkT)
            nc.scalar.dma_start(wv_sb[:], wvT)
            nc.scalar.dma_start(wq_sb[:], wqT)
            nc.scalar.dma_start(wo_sb[:], woT)
            bq_sb = const.tile([P, 2], F32, tag="bq")
            nc.scalar.dma_start(bq_sb[:], bq_v)
            ones_sb = const.tile([1, DK], BF16, tag="ones")
            nc.scalar.dma_start(ones_sb[:], ones_d[None, 0:DK])

            kt_sb = const.tile([P, 2, N], BF16, tag="kt")           # K^T
            v_sb = const.tile([P, NKT, HG, DK + 1], BF16, tag="v")  # [V_h | 1]
            nc.scalar.dma_start(
                v_sb[:, :, :, DK].rearrange("p a b -> p (a b)"),
                ones_d[:, None].to_broadcast((P, NKT * HG)),
            )

            qts = {}
            o_sb = {}
            for c in range(NQB):
                qts[c] = const.tile([P, 2, QB], BF16, tag=f"qt{c}", name=f"qt_{c}")
                o_sb[c] = const.tile([P, 2, QB], BF16, tag=f"o{c}", name=f"o_{c}")

            # x DMAs up front, in consumption order
            xks, xqs = {}, {}
            for c in range(NQB):
                xks[c] = xin.tile([P, KT, QB], BF16, tag="xk", name=f"xk_{c}")
                nc.sync.dma_start(xks[c][:], xkT[c])
            xvs = {}
            for nt in range(NKT):
                xvs[nt] = xvp.tile([P, KT, P], BF16, tag="xv", name=f"xv_{nt}",
                                   bufs=4)
                nc.sync.dma_start(xvs[nt][:], xvT[nt])
            for c in range(NQB):
                xqs[c] = xin.tile([P, KT, QB], BF16, tag="xq", name=f"xq_{c}")
                nc.sync.dma_start(xqs[c][:], xqT[c])

            # ---------------- emission helpers ----------------
            def k_proj(c, j):
                ps = aux.tile([P, QB], F32, tag="aux", name=f"kps_{c}_{j}")
                for k in range(KT):
                    nc.tensor.matmul(
                        ps[:], wk_sb[:, k, j * P:(j + 1) * P], xks[c][:, k, :],
                        start=(k == 0), stop=(k == KT - 1))
                nc.vector.tensor_copy(kt_sb[:, j, c * QB:(c + 1) * QB], ps[:])

            def q_proj(c, j):
                ps = aux.tile([P, QB], F32, tag="aux", name=f"qps_{c}_{j}")
                for k in range(KT):
                    nc.tensor.matmul(
                        ps[:], wq_sb[:, k, j * P:(j + 1) * P], xqs[c][:, k, :],
                        start=(k == 0), stop=(k == KT - 1))
                nc.vector.tensor_tensor(
                    qts[c][:, j, :], ps[:],
                    bq_sb[:, j, None].to_broadcast((P, QB)), ADD)

            def v_proj(nt):
                ps = aux.tile([P, QB], F32, tag="aux", name=f"vps_{nt}")
                for k in range(KT):
                    nc.tensor.matmul(ps[:, 0:DG], xvs[nt][:, k, :], wv_sb[:, k, :],
                                     start=(k == 0), stop=(k == KT - 1))
                nc.vector.tensor_copy(
                    v_sb[:, nt, :, 0:DK],
                    ps[:, 0:DG].rearrange("p (h d) -> p h d", h=HG))

            def o_proj(c, m):
                yps = aux.tile([P, QB], F32, tag="aux", name=f"yps_{c}_{m}")
                for j in range(2):
                    nc.tensor.matmul(
                        yps[:], wo_sb[:, j, m * P:(m + 1) * P], o_sb[c][:, j, :],
                        start=(j == 0), stop=(j == 1))
                y_sb = smal.tile([P, QB], F32, tag="y", name=f"y_{c}_{m}")
                nc.vector.tensor_copy(y_sb[:], yps[:])
                nc.gpsimd.dma_start(
                    yT_d[m * P:(m + 1) * P, c * QB:(c + 1) * QB], y_sb[:])

            # filler queue: PE work units sprinkled into the attention loop
            filler = []
            for c in range(1, NQB):
                filler.append(lambda c=c, j=0: q_proj(c, j))
                filler.append(lambda c=c, j=1: q_proj(c, j))

            def pop_filler(k=1):
                for _ in range(k):
                    if filler:
                        filler.pop(0)()

            def emit_av(c, pair, avs, ats, kt):
                for hp in range(2):
                    h = 2 * pair + hp
                    nc.tensor.matmul(
                        avs[hp][:], v_sb[:, kt, h, :], ats[kt][:, hp, :],
                        start=(kt == 0), stop=(kt == NKT - 1))

            # ---- phase A: kT then V projections ----
            for c in range(NQB):
                for j in range(2):
                    k_proj(c, j)
            for nt in range(NKT):
                v_proj(nt)
            q_proj(0, 0)
            q_proj(0, 1)

            # ---- attention ----
            dbg_at_sb = {}
            for c in range(NQB):
                for pair in range(2):
                    avs = [avp.tile([DK + 1, QB], F32, tag="av",
                                    name=f"av_{c}_{pair}_{hp}")
                           for hp in range(2)]
                    ats = {}
                    for kt in range(NKT):
                        stb = stp.tile([P, 2, QB], F32, tag="st",
                                       name=f"st_{c}_{pair}_{kt}")
                        for hp in range(2):
                            p0 = DK * hp
                            nc.tensor.matmul(
                                stb[:, hp, :],
                                kt_sb[p0:p0 + DK, pair, kt * P:(kt + 1) * P],
                                qts[c][p0:p0 + DK, pair, :],
                                start=True, stop=True)
                        at = atp.tile([P, 2, QB], BF16, tag="at",
                                      name=f"at_{c}_{pair}_{kt}")
                        nc.scalar.activation(
                            at.rearrange("p a b -> p (a b)"),
                            stb.rearrange("p a b -> p (a b)"), EXP, scale=0.125)
                        ats[kt] = at
                        if DEBUG_DUMP and c == 0 and pair == 0 and kt == 0:
                            snap = const.tile([P, 2, QB], F32, tag="dbgsnap")
                            nc.vector.tensor_copy(
                                snap.rearrange("p a b -> p (a b)"),
                                at.rearrange("p a b -> p (a b)"))
                            dbg_at_sb[0] = snap
                        # AV lags one k-tile so exp latency is hidden
                        if kt > 0:
                            emit_av(c, pair, avs, ats, kt - 1)
                        if kt in (2, 5, 8, 11, 14):
                            pop_filler()
                    emit_av(c, pair, avs, ats, NKT - 1)

                    # softmax normalization: 1/Z broadcast via PE outer
                    # product, multiply on DVE
                    if DEBUG_DUMP and c == 0 and pair == 0:
                        snap_av = const.tile([P, QB], F32, tag="dbgav")
                        nc.vector.tensor_copy(snap_av[0:DK + 1, :],
                                              avs[0][0:DK + 1, :])
                        dbg_at_sb["av"] = snap_av
                    for hp in range(2):
                        zr = smal.tile([1, QB], F32, tag="zr",
                                       name=f"zr_{c}_{pair}_{hp}")
                        nc.vector.tensor_copy(zr[:], avs[hp][DK:DK + 1, :])
                        rzf = smal.tile([1, QB], F32, tag="rzf",
                                        name=f"rzf_{c}_{pair}_{hp}")
                        nc.vector.reciprocal_approx_fast(rzf[:], zr[:])
                        zbc = smal.tile([DK, QB], F32, tag="zbc",
                                        name=f"zbc_{c}_{pair}_{hp}")
                        nc.gpsimd.partition_broadcast(zbc[:], rzf[:],
                                                      channels=DK)
                        if DEBUG_DUMP and c == 0 and pair == 0 and hp == 0:
                            snap_bc = const.tile([P, QB], F32, tag="dbgbc")
                            nc.vector.tensor_copy(snap_bc[0:DK, :], zbc[:])
                            dbg_at_sb["bc"] = snap_bc
                        nc.vector.tensor_tensor(
                            o_sb[c][DK * hp:DK * (hp + 1), pair, :],
                            avs[hp][0:DK, :], zbc[:], MULT)

                # out-projection of this chunk becomes filler for the next
                for m in range(8):
                    filler.append(lambda c=c, m=m: o_proj(c, m))

            # drain remaining filler (last chunk's out-projection)
            pop_filler(len(filler))

            if DEBUG_DUMP:
                for nm, sb, dr in (("kt", kt_sb, dbg_kt), ("v", v_sb, dbg_v),
                                   ("qt", qts[0], dbg_qt), ("o", o_sb[0], dbg_o),
                                   ("at", dbg_at_sb[0], dbg_at)):
                    f = smal.tile(list(sb.shape), F32, tag=f"dbg{nm}",
                                  name=f"dbgt_{nm}")
                    nc.vector.tensor_copy(
                        f.rearrange("p a b c -> p (a b c)") if len(sb.shape) == 4
                        else f.rearrange("p a b -> p (a b)"),
                        sb.rearrange("p a b c -> p (a b c)") if len(sb.shape) == 4
                        else sb.rearrange("p a b -> p (a b)"))
                    nc.sync.dma_start(dr, f[:])
                nc.sync.dma_start(dbg_av, dbg_at_sb["av"][:])
                nc.sync.dma_start(dbg_bc, dbg_at_sb["bc"][:])

    nc.compile()
    return nc


def get_program():
    global _PROGRAM
    if _PROGRAM is None:
        _PROGRAM = _build_program()
    return _PROGRAM


def _tile_xT(x, nblk, width):
    # x [n, 1024] f32 -> x^T bf16 tiled [nblk, 128 p, 8 k, width]
    xt = np.ascontiguousarray(x.T).astype(NP_BF16)       # [1024, n]
    return np.ascontiguousarray(
        xt.reshape(KT, P, nblk, width).transpose(2, 1, 0, 3))


def _tile_w(w_rows):
    # w_rows [256, 1024] (= W[g-slice]) -> W^T bf16 tiled [128 p, 8 k, 256]
    return np.ascontiguousarray(
        w_rows.T.astype(NP_BF16).reshape(KT, P, DG).transpose(1, 0, 2))


def make_in_maps(queries, keys, values, Wq, bq, Wk, bk, Wv, bv, Wo, bo):
    """Build per-core input dicts. Core c handles batch c//4, head group c%4."""
    f32 = np.float32
    xT = {}
    for ib in range(B):
        xT[ib] = (
            _tile_xT(np.asarray(queries[ib], f32), NQB, QB),
            _tile_xT(np.asarray(keys[ib], f32), NQB, QB),
            _tile_xT(np.asarray(values[ib], f32), NKT, P),
        )
    ones = np.ones((P,), NP_BF16)
    in_maps = []
    for core in range(8):
        ib, g = core // G, core % G
        sl = slice(g * DG, (g + 1) * DG)
        in_maps.append({
            "xqT": xT[ib][0], "xkT": xT[ib][1], "xvT": xT[ib][2],
            "wqT": _tile_w(Wq[sl, :]),
            "wkT": _tile_w(Wk[sl, :]),
            "wvT": _tile_w(Wv[sl, :]),
            "woT": np.ascontiguousarray(
                Wo[:, sl].T.astype(NP_BF16).reshape(2, P, D_MODEL)
                .transpose(1, 0, 2)),
            "bq_s": np.ascontiguousarray(bq[sl]).astype(f32),
            "ones_c": ones,
        })
    return in_maps


def gather_output(results, Wo, bv, bo):
    out = np.zeros((B, N, D_MODEL), np.float32)
    for core in range(8):
        out[core // G] += results[core]["yT"].T
    # K bias is softmax-invariant (per-query constant logit shift); V bias
    # commutes through normalized attention into a constant output offset.
    const = bo.astype(np.float64) + Wo.astype(np.float64) @ bv.astype(np.float64)
    out += const[None, None, :].astype(np.float32)
    return out


def _run(inputs, trace=False, **spmd_kwargs):
    nc = get_program()
    in_maps = make_in_maps(**inputs)
    res = run_bass_kernel_spmd(nc, in_maps, core_ids=list(range(8)),
                               trace=trace, **spmd_kwargs)
    return gather_output(res.results, inputs["Wo"], inputs["bv"],
                         inputs["bo"]), res


def kernel(**inputs) -> np.ndarray:
    out, _ = _run(inputs, trace=False)
    return out


# revision 32
# speedup vs baseline: 1.6537x; 1.2812x over previous
"""Multi-head attention (b=2, n=2048, d_model=1024, H=16, d_k=d_v=64) on 8
Trainium2 NeuronCores.

Sharding: 8 cores = 2 (batch) x 4 (head groups of 4 heads).  Each core
computes, for its batch ib and head group g (heads as 2 pairs x 2 hp):

    qT/kT projections   qT = Wq_g @ x^T            [256, 2048]
    v projection        V  = x @ Wv_g^T            [2048, 256]
    per head: S^T = K_h Q_h^T (kpos on partitions), A^T = exp(S^T/8),
              O^T|Z = [V_h|1]^T A^T  (PSUM row 64 gives softmax denom Z)
    normalize O^T by 1/Z (Z broadcast across partitions via a tiny
    ones-outer-product matmul on the PE -- no DRAM roundtrip),
    out-projection Y^T = Wo_g @ O_cat^T            [1024, 2048]

Host sums the 4 per-group partial Y^T per batch and adds bo + Wo@bv
(the V bias commutes through normalized attention; the K bias is a
per-query constant shift of the logits, to which softmax is exactly
invariant, so both are folded out of the device program).

All matmul operands are bf16 (PSUM accumulation stays fp32): this halves
input DMA, enables Fast Weight Load on the PE (2x faster LDWEIGHTS), and
doubles DVE throughput where it applies.  exp is evaluated on fp32 PSUM
logits.  Softmax skips max-subtraction: scores*scale are ~N(0,1).

The PE instruction stream is software-pipelined: within the attention
inner loop the AV matmuls lag the S matmuls by one k-tile so the ACT
(exp) stream is hidden, and projection / out-projection matmuls are
sprinkled between attention tiles as filler so the PE never idles long
enough for the HAM clock gate to re-throttle.
"""

import numpy as np
from contextlib import ExitStack

import ml_dtypes

import concourse.bass as bass
import concourse.mybir as mybir
import concourse.tile as tile
from concourse import bacc
from concourse.bass_utils import run_bass_kernel_spmd

F32 = mybir.dt.float32
BF16 = mybir.dt.bfloat16
NP_BF16 = ml_dtypes.bfloat16
EXP = mybir.ActivationFunctionType.Exp
ADD = mybir.AluOpType.add
MULT = mybir.AluOpType.mult

D_MODEL = 1024
H = 16
DK = 64
B = 2
N = 2048           # nq = nk
G = 4              # head groups (cores per batch)
HG = H // G        # heads per group = 4
DG = HG * DK       # 256 group dims
KT = 8             # D_MODEL / 128 contraction tiles
NKT = N // 128     # 16 k-tiles in attention
QB = 512           # attention q-block
NQB = N // QB      # 4 q-blocks
P = 128

_PROGRAM = None
DEBUG_DUMP = False


def _build_program():
    nc = bacc.Bacc("TRN2", target_bir_lowering=False, debug=False, num_devices=8)

    # pre-tiled on host (bf16): per-partition lines are contiguous reads
    xqT = nc.dram_tensor("xqT", [NQB, P, KT, QB], BF16, kind="ExternalInput").ap()
    xkT = nc.dram_tensor("xkT", [NQB, P, KT, QB], BF16, kind="ExternalInput").ap()
    xvT = nc.dram_tensor("xvT", [NKT, P, KT, P], BF16, kind="ExternalInput").ap()
    wqT = nc.dram_tensor("wqT", [P, KT, DG], BF16, kind="ExternalInput").ap()
    wkT = nc.dram_tensor("wkT", [P, KT, DG], BF16, kind="ExternalInput").ap()
    wvT = nc.dram_tensor("wvT", [P, KT, DG], BF16, kind="ExternalInput").ap()
    woT = nc.dram_tensor("woT", [P, 2, D_MODEL], BF16, kind="ExternalInput").ap()
    bq_d = nc.dram_tensor("bq_s", [DG], F32, kind="ExternalInput").ap()
    yT_d = nc.dram_tensor("yT", [D_MODEL, N], F32, kind="ExternalOutput").ap()
    if DEBUG_DUMP:
        dbg_kt = nc.dram_tensor("dbg_kt", [P, 2, N], F32, kind="ExternalOutput").ap()
        dbg_v = nc.dram_tensor("dbg_v", [P, NKT, HG, DK + 1], F32,
                               kind="ExternalOutput").ap()
        dbg_qt = nc.dram_tensor("dbg_qt", [P, 2, QB], F32, kind="ExternalOutput").ap()
        dbg_o = nc.dram_tensor("dbg_o", [P, 2, QB], F32, kind="ExternalOutput").ap()
        dbg_at = nc.dram_tensor("dbg_at", [P, 2, QB], F32, kind="ExternalOutput").ap()
        dbg_av = nc.dram_tensor("dbg_av", [P, QB], F32, kind="ExternalOutput").ap()
        dbg_bc = nc.dram_tensor("dbg_bc", [P, QB], F32, kind="ExternalOutput").ap()

    bq_v = bq_d.rearrange("(j p) -> p j", p=P)        # [128, 2]

    with tile.TileContext(nc) as tc:
        with ExitStack() as ctx:
            const = ctx.enter_context(tc.tile_pool(name="const", bufs=1))
            xin = ctx.enter_context(tc.tile_pool(name="xin", bufs=2))
            xvp = ctx.enter_context(tc.tile_pool(name="xvp", bufs=3))
            atp = ctx.enter_context(tc.tile_pool(name="atp", bufs=3))
            smal = ctx.enter_context(tc.tile_pool(name="smal", bufs=2))
            stp = ctx.enter_context(tc.tile_pool(name="stp", bufs=2, space="PSUM"))
            avp = ctx.enter_context(tc.tile_pool(name="avp", bufs=2, space="PSUM"))
            aux = ctx.enter_context(tc.tile_pool(name="aux", bufs=2, space="PSUM"))

            # ---- constants: weights on the ACT queue (idle at start),
            # x chunks on SP, xv on DVE ----
            wk_sb = const.tile([P, KT, DG], BF16, tag="wk")
            wv_sb = const.tile([P, KT, DG], BF16, tag="wv")
            wq_sb = const.tile([P, KT, DG], BF16, tag="wq")
            wo_sb = const.tile([P, 2, D_MODEL], BF16, tag="wo")
            nc.scalar.dma_start(wk_sb[:], wkT)
            nc.scalar.dma_start(wv_sb[:], wvT)
            nc.scalar.dma_start(wq_sb[:], wqT)
            nc.scalar.dma_start(wo_sb[:], woT)
            bq_sb = const.tile([P, 2], F32, tag="bq")
            nc.scalar.dma_start(bq_sb[:], bq_v)

            kt_sb = const.tile([P, 2, N], BF16, tag="kt")           # K^T
            v_sb = const.tile([P, NKT, HG, DK + 1], BF16, tag="v")  # [V_h | 1]
            # whole-tile memset to 1.0; V-proj copies overwrite cols 0:64,
            # leaving the denominator ones-column.  (A strided ones DMA here
            # generates thousands of 2-byte descriptors and stalls the DMA
            # engines for ~50us.)
            nc.vector.memset(
                v_sb.rearrange("p a b c -> p (a b c)"), 1.0)

            qts = {}
            o_sb = {}
            for c in range(NQB):
                qts[c] = const.tile([P, 2, QB], BF16, tag=f"qt{c}", name=f"qt_{c}")
                o_sb[c] = const.tile([P, 2, QB], BF16, tag=f"o{c}", name=f"o_{c}")

            # x DMAs up front, in consumption order
            xks, xqs = {}, {}
            for c in range(NQB):
                xks[c] = xin.tile([P, KT, QB], BF16, tag="xk", name=f"xk_{c}",
                                  bufs=4)
                nc.sync.dma_start(xks[c][:], xkT[c])
            xvs = {}
            for nt in range(NKT):
                xvs[nt] = xvp.tile([P, KT, P], BF16, tag="xv", name=f"xv_{nt}",
                                   bufs=8)
                nc.sync.dma_start(xvs[nt][:], xvT[nt])
            for c in range(NQB):
                xqs[c] = xin.tile([P, KT, QB], BF16, tag="xq", name=f"xq_{c}",
                                  bufs=4)
                nc.sync.dma_start(xqs[c][:], xqT[c])

            # ---------------- emission helpers ----------------
            def k_proj(c, j):
                ps = aux.tile([P, QB], F32, tag="aux", name=f"kps_{c}_{j}")
                for k in range(KT):
                    nc.tensor.matmul(
                        ps[:], wk_sb[:, k, j * P:(j + 1) * P], xks[c][:, k, :],
                        start=(k == 0), stop=(k == KT - 1))
                nc.vector.tensor_copy(kt_sb[:, j, c * QB:(c + 1) * QB], ps[:])

            def q_proj(c, j):
                ps = aux.tile([P, QB], F32, tag="aux", name=f"qps_{c}_{j}")
                for k in range(KT):
                    nc.tensor.matmul(
                        ps[:], wq_sb[:, k, j * P:(j + 1) * P], xqs[c][:, k, :],
                        start=(k == 0), stop=(k == KT - 1))
                nc.vector.tensor_tensor(
                    qts[c][:, j, :], ps[:],
                    bq_sb[:, j, None].to_broadcast((P, QB)), ADD)

            def v_proj(nt):
                ps = aux.tile([P, QB], F32, tag="aux", name=f"vps_{nt}")
                for k in range(KT):
                    nc.tensor.matmul(ps[:, 0:DG], xvs[nt][:, k, :], wv_sb[:, k, :],
                                     start=(k == 0), stop=(k == KT - 1))
                nc.vector.tensor_copy(
                    v_sb[:, nt, :, 0:DK],
                    ps[:, 0:DG].rearrange("p (h d) -> p h d", h=HG))

            def o_proj(c, m):
                yps = aux.tile([P, QB], F32, tag="aux", name=f"yps_{c}_{m}")
                for j in range(2):
                    nc.tensor.matmul(
                        yps[:], wo_sb[:, j, m * P:(m + 1) * P], o_sb[c][:, j, :],
                        start=(j == 0), stop=(j == 1))
                y_sb = smal.tile([P, QB], F32, tag="y", name=f"y_{c}_{m}")
                nc.vector.tensor_copy(y_sb[:], yps[:])
                nc.gpsimd.dma_start(
                    yT_d[m * P:(m + 1) * P, c * QB:(c + 1) * QB], y_sb[:])

            # filler queue: PE work units sprinkled into the attention loop
            filler = []
            for c in range(1, NQB):
                filler.append(lambda c=c, j=0: q_proj(c, j))
                filler.append(lambda c=c, j=1: q_proj(c, j))

            def pop_filler(k=1):
                for _ in range(k):
                    if filler:
                        filler.pop(0)()

            def emit_av(c, pair, avs, ats, kt):
                for hp in range(2):
                    h = 2 * pair + hp
                    nc.tensor.matmul(
                        avs[hp][:], v_sb[:, kt, h, :], ats[kt][:, hp, :],
                        start=(kt == 0), stop=(kt == NKT - 1))

            # ---- phase A: kT then V projections ----
            for c in range(NQB):
                for j in range(2):
                    k_proj(c, j)
            for nt in range(NKT):
                v_proj(nt)
            q_proj(0, 0)
            q_proj(0, 1)

            # ---- attention ----
            dbg_at_sb = {}
            for c in range(NQB):
                for pair in range(2):
                    avs = [avp.tile([DK + 1, QB], F32, tag="av",
                                    name=f"av_{c}_{pair}_{hp}")
                           for hp in range(2)]
                    ats = {}
                    for kt in range(NKT):
                        stb = stp.tile([P, 2, QB], F32, tag="st",
                                       name=f"st_{c}_{pair}_{kt}")
                        for hp in range(2):
                            p0 = DK * hp
                            nc.tensor.matmul(
                                stb[:, hp, :],
                                kt_sb[p0:p0 + DK, pair, kt * P:(kt + 1) * P],
                                qts[c][p0:p0 + DK, pair, :],
                                start=True, stop=True)
                        at = atp.tile([P, 2, QB], BF16, tag="at",
                                      name=f"at_{c}_{pair}_{kt}")
                        nc.scalar.activation(
                            at.rearrange("p a b -> p (a b)"),
                            stb.rearrange("p a b -> p (a b)"), EXP, scale=0.125)
                        ats[kt] = at
                        if DEBUG_DUMP and c == 0 and pair == 0 and kt == 0:
                            snap = const.tile([P, 2, QB], F32, tag="dbgsnap")
                            nc.vector.tensor_copy(
                                snap.rearrange("p a b -> p (a b)"),
                                at.rearrange("p a b -> p (a b)"))
                            dbg_at_sb[0] = snap
                        # AV lags one k-tile so exp latency is hidden
                        if kt > 0:
                            emit_av(c, pair, avs, ats, kt - 1)
                        if kt in (2, 5, 8, 11, 14):
                            pop_filler()
                    emit_av(c, pair, avs, ats, NKT - 1)

                    # softmax normalization: 1/Z broadcast via PE outer
                    # product, multiply on DVE
                    if DEBUG_DUMP and c == 0 and pair == 0:
                        snap_av = const.tile([P, QB], F32, tag="dbgav")
                        nc.vector.tensor_copy(snap_av[0:DK + 1, :],
                                              avs[0][0:DK + 1, :])
                        dbg_at_sb["av"] = snap_av
                    for hp in range(2):
                        zr = smal.tile([1, QB], F32, tag="zr",
                                       name=f"zr_{c}_{pair}_{hp}")
                        nc.vector.tensor_copy(zr[:], avs[hp][DK:DK + 1, :])
                        rzf = smal.tile([1, QB], F32, tag="rzf",
                                        name=f"rzf_{c}_{pair}_{hp}")
                        nc.vector.reciprocal_approx_fast(rzf[:], zr[:])
                        zbc = smal.tile([DK, QB], F32, tag="zbc",
                                        name=f"zbc_{c}_{pair}_{hp}")
                        nc.gpsimd.partition_broadcast(zbc[:], rzf[:],
                                                      channels=DK)
                        if DEBUG_DUMP and c == 0 and pair == 0 and hp == 0:
                            snap_bc = const.tile([P, QB], F32, tag="dbgbc")
                            nc.vector.tensor_copy(snap_bc[0:DK, :], zbc[:])
                            dbg_at_sb["bc"] = snap_bc
                        nc.vector.tensor_tensor(
                            o_sb[c][DK * hp:DK * (hp + 1), pair, :],
                            avs[hp][0:DK, :], zbc[:], MULT)

                # out-projection of this chunk becomes filler for the next
                for m in range(8):
                    filler.append(lambda c=c, m=m: o_proj(c, m))

            # drain remaining filler (last chunk's out-projection)
            pop_filler(len(filler))

            if DEBUG_DUMP:
                for nm, sb, dr in (("kt", kt_sb, dbg_kt), ("v", v_sb, dbg_v),
                                   ("qt", qts[0], dbg_qt), ("o", o_sb[0], dbg_o),
                                   ("at", dbg_at_sb[0], dbg_at)):
                    f = smal.tile(list(sb.shape), F32, tag=f"dbg{nm}",
                                  name=f"dbgt_{nm}")
                    nc.vector.tensor_copy(
                        f.rearrange("p a b c -> p (a b c)") if len(sb.shape) == 4
                        else f.rearrange("p a b -> p (a b)"),
                        sb.rearrange("p a b c -> p (a b c)") if len(sb.shape) == 4
                        else sb.rearrange("p a b -> p (a b)"))
                    nc.sync.dma_start(dr, f[:])
                nc.sync.dma_start(dbg_av, dbg_at_sb["av"][:])
                nc.sync.dma_start(dbg_bc, dbg_at_sb["bc"][:])

    nc.compile()
    return nc


def get_program():
    global _PROGRAM
    if _PROGRAM is None:
        _PROGRAM = _build_program()
    return _PROGRAM


def _tile_xT(x, nblk, width):
    # x [n, 1024] f32 -> x^T bf16 tiled [nblk, 128 p, 8 k, width]
    xt = np.ascontiguousarray(x.T).astype(NP_BF16)       # [1024, n]
    return np.ascontiguousarray(
        xt.reshape(KT, P, nblk, width).transpose(2, 1, 0, 3))


def _tile_w(w_rows):
    # w_rows [256, 1024] (= W[g-slice]) -> W^T bf16 tiled [128 p, 8 k, 256]
    return np.ascontiguousarray(
        w_rows.T.astype(NP_BF16).reshape(KT, P, DG).transpose(1, 0, 2))


def make_in_maps(queries, keys, values, Wq, bq, Wk, bk, Wv, bv, Wo, bo):
    """Build per-core input dicts. Core c handles batch c//4, head group c%4."""
    f32 = np.float32
    xT = {}
    for ib in range(B):
        xT[ib] = (
            _tile_xT(np.asarray(queries[ib], f32), NQB, QB),
            _tile_xT(np.asarray(keys[ib], f32), NQB, QB),
            _tile_xT(np.asarray(values[ib], f32), NKT, P),
        )
    in_maps = []
    for core in range(8):
        ib, g = core // G, core % G
        sl = slice(g * DG, (g + 1) * DG)
        in_maps.append({
            "xqT": xT[ib][0], "xkT": xT[ib][1], "xvT": xT[ib][2],
            "wqT": _tile_w(Wq[sl, :]),
            "wkT": _tile_w(Wk[sl, :]),
            "wvT": _tile_w(Wv[sl, :]),
            "woT": np.ascontiguousarray(
                Wo[:, sl].T.astype(NP_BF16).reshape(2, P, D_MODEL)
                .transpose(1, 0, 2)),
            "bq_s": np.ascontiguousarray(bq[sl]).astype(f32),
        })
    return in_maps


def gather_output(results, Wo, bv, bo):
    out = np.zeros((B, N, D_MODEL), np.float32)
    for core in range(8):
        out[core // G] += results[core]["yT"].T
    # K bias is softmax-invariant (per-query constant logit shift); V bias
    # commutes through normalized attention into a constant output offset.
    const = bo.astype(np.float64) + Wo.astype(np.float64) @ bv.astype(np.float64)
    out += const[None, None, :].astype(np.float32)
    return out


def _run(inputs, trace=False, **spmd_kwargs):
    nc = get_program()
    in_maps = make_in_maps(**inputs)
    res = run_bass_kernel_spmd(nc, in_maps, core_ids=list(range(8)),
                               trace=trace, **spmd_kwargs)
    return gather_output(res.results, inputs["Wo"], inputs["bv"],
                         inputs["bo"]), res


def kernel(**inputs) -> np.ndarray:
    out, _ = _run(inputs, trace=False)
    return out
